# revision 1
# baseline (speedup 1.0000x reference)
"""AFNO1D Trainium2 kernel: FFT->block-MLP->softshrink->IFFT->residual.

Strategy: the FFT along C is linear, so it is fused into the layer-1
weights on the host (W1_eff = DFT_block @ w1), and the IFFT becomes two
dense [1024x1024] matmuls after softshrink. Everything on-chip is plain
matmul + activation work, computed in a channel-major ("transposed")
layout so the contraction dim always sits on SBUF partitions.

Data-parallel over B=8: core b handles x[b] ([4096, 1024]); params are
replicated. No collectives. Host transposes each shard to [1024, 4096]
so no on-chip transposes are needed, and transposes the result back.

Compute dtype: bf16 operands with fp32 PSUM accumulation (validated:
rel err ~2e-4 vs fp32 reference, the residual dominates the output).
"""

from contextlib import ExitStack

import numpy as np
import ml_dtypes

import concourse.bass as bass
import concourse.mybir as mybir
import concourse.tile as tile
from concourse import bacc
from concourse.bass_utils import run_bass_kernel_spmd

HIDDEN = 1024
NB = 8          # channel blocks
BS = 128        # block size
LAM = 0.01
N_CORES = 8
NROWS = 4096    # rows (sequence positions) per core
R = 512         # rows per chunk
NCHUNK = NROWS // R

F32 = mybir.dt.float32
BF16 = mybir.dt.bfloat16
RELU = mybir.ActivationFunctionType.Relu

_GRAPH_CACHE = {}


def _build_graph():
    if "nc" in _GRAPH_CACHE:
        return _GRAPH_CACHE["nc"]

    nc = bacc.Bacc("TRN2", target_bir_lowering=False, debug=False,
                   num_devices=N_CORES)

    xt = nc.dram_tensor("xt", [NB, BS, NROWS], F32, kind="ExternalInput").ap()
    w1r = nc.dram_tensor("w1r", [NB, BS, HIDDEN], BF16, kind="ExternalInput").ap()
    w1i = nc.dram_tensor("w1i", [NB, BS, HIDDEN], BF16, kind="ExternalInput").ap()
    gr = nc.dram_tensor("gr", [NB, BS, HIDDEN], BF16, kind="ExternalInput").ap()
    gi = nc.dram_tensor("gi", [NB, BS, HIDDEN], BF16, kind="ExternalInput").ap()
    w20 = nc.dram_tensor("w20", [NB, BS, BS], BF16, kind="ExternalInput").ap()
    w21 = nc.dram_tensor("w21", [NB, BS, BS], BF16, kind="ExternalInput").ap()
    w21n = nc.dram_tensor("w21n", [NB, BS, BS], BF16, kind="ExternalInput").ap()
    b1r = nc.dram_tensor("b1r", [BS, NB], F32, kind="ExternalInput").ap()
    b1i = nc.dram_tensor("b1i", [BS, NB], F32, kind="ExternalInput").ap()
    b2rm = nc.dram_tensor("b2rm", [BS, NB], F32, kind="ExternalInput").ap()
    b2rp = nc.dram_tensor("b2rp", [BS, NB], F32, kind="ExternalInput").ap()
    b2im = nc.dram_tensor("b2im", [BS, NB], F32, kind="ExternalInput").ap()
    b2ip = nc.dram_tensor("b2ip", [BS, NB], F32, kind="ExternalInput").ap()
    out = nc.dram_tensor("out", [NB, BS, NROWS], F32, kind="ExternalOutput").ap()

    with tile.TileContext(nc) as tc, ExitStack() as ctx:
        wpool = ctx.enter_context(tc.tile_pool(name="weights", bufs=1))
        w1r_sb = wpool.tile([BS, NB, HIDDEN], BF16, tag="w1r", name="w1r_sb")
        w1i_sb = wpool.tile([BS, NB, HIDDEN], BF16, tag="w1i", name="w1i_sb")
        gr_sb = wpool.tile([BS, NB, HIDDEN], BF16, tag="gr", name="gr_sb")
        gi_sb = wpool.tile([BS, NB, HIDDEN], BF16, tag="gi", name="gi_sb")
        for ci in range(NB):
            nc.sync.dma_start(out=w1r_sb[:, ci, :], in_=w1r[ci])
            nc.sync.dma_start(out=w1i_sb[:, ci, :], in_=w1i[ci])
            nc.sync.dma_start(out=gr_sb[:, ci, :], in_=gr[ci])
            nc.sync.dma_start(out=gi_sb[:, ci, :], in_=gi[ci])
        w20_sb = wpool.tile([BS, NB, BS], BF16, tag="w20", name="w20_sb")
        w21_sb = wpool.tile([BS, NB, BS], BF16, tag="w21", name="w21_sb")
        w21n_sb = wpool.tile([BS, NB, BS], BF16, tag="w21n", name="w21n_sb")
        for kb in range(NB):
            nc.sync.dma_start(out=w20_sb[:, kb, :], in_=w20[kb])
            nc.sync.dma_start(out=w21_sb[:, kb, :], in_=w21[kb])
            nc.sync.dma_start(out=w21n_sb[:, kb, :], in_=w21n[kb])
        bias_tiles = {}
        for nm, ap in (("b1r", b1r), ("b1i", b1i), ("b2rm", b2rm),
                       ("b2rp", b2rp), ("b2im", b2im), ("b2ip", b2ip)):
            t = wpool.tile([BS, NB], F32, tag=nm, name=f"{nm}_sb")
            nc.sync.dma_start(out=t[:], in_=ap[:])
            bias_tiles[nm] = t

        xpool = ctx.enter_context(tc.tile_pool(name="xin", bufs=2))
        bfpool = ctx.enter_context(tc.tile_pool(name="bf", bufs=2))
        opool = ctx.enter_context(tc.tile_pool(name="acts", bufs=1))
        outpool = ctx.enter_context(tc.tile_pool(name="outs", bufs=1))
        ppool = ctx.enter_context(tc.tile_pool(name="psum", bufs=4, space="PSUM"))

        for ch in range(NCHUNK):
            r0 = ch * R
            xt_f = xpool.tile([BS, NB, R], F32, tag="xt_f", name=f"xt_f{ch}")
            for ci in range(NB):
                nc.sync.dma_start(out=xt_f[:, ci, :], in_=xt[ci, :, r0:r0 + R])
            xt_b = bfpool.tile([BS, NB, R], BF16, tag="xt_b", name=f"xt_b{ch}")
            nc.vector.tensor_copy(xt_b[:], xt_f[:])

            # layer 1 (FFT fused): o1 = relu(W1_eff^T @ xt + b1), channel-major
            o1r = opool.tile([BS, NB, R], BF16, tag="o1r", name=f"o1r{ch}")
            o1i = opool.tile([BS, NB, R], BF16, tag="o1i", name=f"o1i{ch}")
            for ro in range(NB):
                pr = ppool.tile([BS, R], F32, tag="pr", name=f"pr{ch}_{ro}")
                pi = ppool.tile([BS, R], F32, tag="pi", name=f"pi{ch}_{ro}")
                for ci in range(NB):
                    nc.tensor.matmul(pr[:], w1r_sb[:, ci, ro * BS:(ro + 1) * BS],
                                     xt_b[:, ci, :], start=(ci == 0), stop=(ci == NB - 1))
                for ci in range(NB):
                    nc.tensor.matmul(pi[:], w1i_sb[:, ci, ro * BS:(ro + 1) * BS],
                                     xt_b[:, ci, :], start=(ci == 0), stop=(ci == NB - 1))
                nc.scalar.activation(o1r[:, ro, :], pr[:], RELU,
                                     bias=bias_tiles["b1r"][:, ro:ro + 1])
                nc.scalar.activation(o1i[:, ro, :], pi[:], RELU,
                                     bias=bias_tiles["b1i"][:, ro:ro + 1])

            # layer 2: block-diagonal complex matmul + softshrink
            # softshrink(q + b2) = relu(q + b2 - lam) - relu(-q - b2 - lam)
            o2r = opool.tile([BS, NB, R], BF16, tag="o2r", name=f"o2r{ch}")
            o2i = opool.tile([BS, NB, R], BF16, tag="o2i", name=f"o2i{ch}")
            for kb in range(NB):
                qr = ppool.tile([BS, R], F32, tag="pr", name=f"qr{ch}_{kb}")
                qi = ppool.tile([BS, R], F32, tag="pi", name=f"qi{ch}_{kb}")
                nc.tensor.matmul(qr[:], w20_sb[:, kb, :], o1r[:, kb, :],
                                 start=True, stop=False)
                nc.tensor.matmul(qr[:], w21n_sb[:, kb, :], o1i[:, kb, :],
                                 start=False, stop=True)
                nc.tensor.matmul(qi[:], w20_sb[:, kb, :], o1i[:, kb, :],
                                 start=True, stop=False)
                nc.tensor.matmul(qi[:], w21_sb[:, kb, :], o1r[:, kb, :],
                                 start=False, stop=True)
                t1 = bfpool.tile([BS, R], BF16, tag="t1", name=f"t1_{ch}_{kb}")
                t2 = bfpool.tile([BS, R], BF16, tag="t2", name=f"t2_{ch}_{kb}")
                nc.scalar.activation(t1[:], qr[:], RELU,
                                     bias=bias_tiles["b2rm"][:, kb:kb + 1])
                nc.scalar.activation(t2[:], qr[:], RELU, scale=-1.0,
                                     bias=bias_tiles["b2rp"][:, kb:kb + 1])
                nc.vector.tensor_sub(o2r[:, kb, :], t1[:], t2[:])
                t3 = bfpool.tile([BS, R], BF16, tag="t3", name=f"t3_{ch}_{kb}")
                t4 = bfpool.tile([BS, R], BF16, tag="t4", name=f"t4_{ch}_{kb}")
                nc.scalar.activation(t3[:], qi[:], RELU,
                                     bias=bias_tiles["b2im"][:, kb:kb + 1])
                nc.scalar.activation(t4[:], qi[:], RELU, scale=-1.0,
                                     bias=bias_tiles["b2ip"][:, kb:kb + 1])
                nc.vector.tensor_sub(o2i[:, kb, :], t3[:], t4[:])

            # layer 3 (IFFT real part) + residual
            out_f = outpool.tile([BS, NB, R], F32, tag="out_f", name=f"out_f{ch}")
            for co in range(NB):
                p3 = ppool.tile([BS, R], F32, tag="pr", name=f"p3_{ch}_{co}")
                for kb in range(NB):
                    nc.tensor.matmul(p3[:], gr_sb[:, kb, co * BS:(co + 1) * BS],
                                     o2r[:, kb, :], start=(kb == 0), stop=False)
                for kb in range(NB):
                    nc.tensor.matmul(p3[:], gi_sb[:, kb, co * BS:(co + 1) * BS],
                                     o2i[:, kb, :], start=False, stop=(kb == NB - 1))
                nc.vector.tensor_add(out_f[:, co, :], p3[:], xt_f[:, co, :])
                nc.sync.dma_start(out=out[co, :, r0:r0 + R], in_=out_f[:, co, :])

    nc.compile()
    _GRAPH_CACHE["nc"] = nc
    return nc


def _build_host_weights(w1, b1, w2, b2):
    C = HIDDEN
    k = np.arange(C)
    c = np.arange(C)
    ph = (np.outer(c, k) % C).astype(np.float64) * (2.0 * np.pi / C)
    s = 1.0 / np.sqrt(C)
    Fr = np.cos(ph) * s        # [c, k]
    Fi = -np.sin(ph) * s
    w1 = np.asarray(w1, np.float64)
    W1r = np.empty((C, C), np.float64)
    W1i = np.empty((C, C), np.float64)
    for kb in range(NB):
        cols = slice(kb * BS, (kb + 1) * BS)
        W1r[:, cols] = Fr[:, cols] @ w1[0, kb] - Fi[:, cols] @ w1[1, kb]
        W1i[:, cols] = Fi[:, cols] @ w1[0, kb] + Fr[:, cols] @ w1[1, kb]
    # IFFT (real part): out = o2r @ Gr + o2i @ Gi, G[k, c]
    Gr = Fr.T.copy()           # cos(2pi k c / C) / sqrt(C)
    Gi = Fi.T.copy()           # -sin(2pi k c / C) / sqrt(C)

    bf = ml_dtypes.bfloat16
    b1 = np.asarray(b1, np.float32)
    b2 = np.asarray(b2, np.float32)
    w2 = np.asarray(w2, np.float32)
    return {
        "w1r": np.ascontiguousarray(W1r.reshape(NB, BS, HIDDEN)).astype(bf),
        "w1i": np.ascontiguousarray(W1i.reshape(NB, BS, HIDDEN)).astype(bf),
        "gr": np.ascontiguousarray(Gr.reshape(NB, BS, HIDDEN)).astype(bf),
        "gi": np.ascontiguousarray(Gi.reshape(NB, BS, HIDDEN)).astype(bf),
        "w20": w2[0].astype(bf),
        "w21": w2[1].astype(bf),
        "w21n": (-w2[1]).astype(bf),
        "b1r": np.ascontiguousarray(b1[0].T),
        "b1i": np.ascontiguousarray(b1[1].T),
        "b2rm": np.ascontiguousarray((b2[0] - LAM).T),
        "b2rp": np.ascontiguousarray((-b2[0] - LAM).T),
        "b2im": np.ascontiguousarray((b2[1] - LAM).T),
        "b2ip": np.ascontiguousarray((-b2[1] - LAM).T),
    }


def _run(x, w1, b1, w2, b2, trace=False):
    nc = _build_graph()
    x = np.asarray(x, np.float32)
    B = x.shape[0]
    weights = _build_host_weights(w1, b1, w2, b2)
    in_maps = []
    for b in range(B):
        m = dict(weights)
        m["xt"] = np.ascontiguousarray(x[b].T).reshape(NB, BS, NROWS)
        in_maps.append(m)
    res = run_bass_kernel_spmd(nc, in_maps, core_ids=list(range(N_CORES)),
                               trace=trace)
    outs = np.empty_like(x)
    for b in range(B):
        outs[b] = res.results[b]["out"].reshape(HIDDEN, NROWS).T
    return outs, res


def kernel(x, w1, b1, w2, b2):
    outs, _ = _run(x, w1, b1, w2, b2, trace=False)
    return outs
